# revision 85
# baseline (speedup 1.0000x reference)
"""DiffTransformer layer on 8 TRN2 NeuronCores.

Sharding: core c = (batch b=c//2, head-group g=c%2). Each core computes
q/k/v projections + differential attention for its 8 heads of its batch
(transposed [feature, seq] layout), a partial out-projection over its
512 attention channels, then pair ReduceScatters ([0,1],[2,3],...) sum
the two head-groups' partials and hand each core a 512-seq shard, on
which it runs the full FFN + residual + final RMSNorm.

vs baseline: bf16 data path everywhere (x/w/q/k/v/e/attn), causal mask
as a -300 additive band folded into the score PSUM via a PE matmul
(exp then yields ~0, no DVE mask), per-(pt,head-pass) attention with
batched 2-head exp from a 2-bank PSUM score tile, softmax denominators
via the va ones-column, division deferred into the subln RMS, all
partition broadcasts done as K=16/K=8 selector matmuls on the PE
(lambda folded into the selector host-side), rsqrt as exp(-.5*ln) to
keep ACT on one table set, wo emitted per D-half so the pair
ReduceScatter fires early, and the FFN h1 contraction split kd0-3 /
kd4-7 so the second RS overlaps the first half of h1.
"""
import os
import sys
import numpy as np

for _p in ("/opt/trn_rl_repo", "/root/.axon_site/_ro/trn_rl_repo"):
    if os.path.isdir(_p) and _p not in sys.path:
        sys.path.append(_p)

B, S, D, H, HD, FF = 4, 1024, 1024, 16, 32, 4096
NCORES = 8
LAMBDA_INIT = 0.8 - 0.6 * float(np.exp(-0.3 * 12))
EPS = 1e-5
SCALE = float(HD) ** -0.5
BAND = -300.0

SWAP16 = [((i + 16) % 32) for i in range(32)]

LAST_RESULT = None  # BassKernelResults of the most recent run (for test.py)
_PROGRAM = {}


def _kts(qc):
    # (k-tile index, diag-band offset or None=full) for a 512-wide q chunk
    if qc == 0:
        return [(0, 0), (1, 128), (2, 256), (3, 384)]
    return [(0, None), (1, None), (2, None), (3, None),
            (4, 0), (5, 128), (6, 256), (7, 384)]


def _build_program():
    import concourse.bacc as bacc
    import concourse.mybir as mybir
    from concourse import tile
    from contextlib import ExitStack

    dt = mybir.dt
    f32, f32r = dt.float32, dt.float32r
    bf16 = dt.bfloat16
    Alu = mybir.AluOpType
    Act = mybir.ActivationFunctionType

    nc = bacc.Bacc("TRN2", target_bir_lowering=False, debug=False,
                   num_devices=NCORES)

    P = 128
    xT = nc.declare_dram_parameter("xT", [D, S], bf16, isOutput=False)
    wqT = nc.declare_dram_parameter("wqT", [D, 512], bf16, isOutput=False)
    wkT = nc.declare_dram_parameter("wkT", [D, 512], bf16, isOutput=False)
    wvT = nc.declare_dram_parameter("wvT", [D, 512], bf16, isOutput=False)
    woT = nc.declare_dram_parameter("woT", [512, D], bf16, isOutput=False)
    w1s = nc.declare_dram_parameter("w1s", [32, P, 1024], bf16, isOutput=False)
    w2T = nc.declare_dram_parameter("w2T", [FF, D], bf16, isOutput=False)
    b1c = nc.declare_dram_parameter("b1c", [P, 32], f32, isOutput=False)
    b2c = nc.declare_dram_parameter("b2c", [P, 8], f32, isOutput=False)
    rmswc = nc.declare_dram_parameter("rmswc", [P, 8], f32, isOutput=False)
    cosT = nc.declare_dram_parameter("cosT", [P, S], f32, isOutput=False)
    sinS = nc.declare_dram_parameter("sinS", [P, S], f32, isOutput=False)
    mdiag = nc.declare_dram_parameter("mdiag", [P, 2 * P], bf16,
                                      isOutput=False)
    hz1l = nc.declare_dram_parameter("hz1l", [P, 128], f32r, isOutput=False)
    hz2 = nc.declare_dram_parameter("hz2", [P, 128], f32r, isOutput=False)
    hselq = nc.declare_dram_parameter("hselq", [P, 32], f32r, isOutput=False)
    hrstd = nc.declare_dram_parameter("hrstd", [8, 512], f32r, isOutput=False)
    outT = nc.declare_dram_parameter("outT", [D, 512], f32, isOutput=True)

    with tile.TileContext(nc) as tc:
        with (
            tc.tile_pool(name="consts", bufs=1) as consts,
            tc.tile_pool(name="dram", bufs=1, space="DRAM") as dram,
        ):
            # ---- constants -------------------------------------------
            md_sb = consts.tile([P, 2 * P], bf16, tag="md")
            hz1l_sb = consts.tile([P, 128], f32r, tag="hz1l")
            hz2_sb = consts.tile([P, 128], f32r, tag="hz2")
            hselq_sb = consts.tile([P, 32], f32r, tag="hselq")
            hrstd_sb = consts.tile([8, 512], f32r, tag="hrstd")
            b1_sb = consts.tile([P, 32], f32, tag="b1")
            b2_sb = consts.tile([P, 8], f32, tag="b2")
            rw_sb = consts.tile([P, 8], f32, tag="rw")
            wo_sb = [consts.tile([P, D], bf16, tag=f"wo{i}", name=f"wo{i}")
                     for i in range(4)]
            nc.sync.dma_start(md_sb[:], mdiag[:])
            nc.sync.dma_start(hz1l_sb[:], hz1l[:])
            nc.sync.dma_start(hz2_sb[:], hz2[:])
            nc.sync.dma_start(hselq_sb[:], hselq[:])
            nc.sync.dma_start(hrstd_sb[:], hrstd[:])
            nc.sync.dma_start(b1_sb[:], b1c[:])
            nc.sync.dma_start(b2_sb[:], b2c[:])
            nc.sync.dma_start(rw_sb[:], rmswc[:])
            for i in range(4):
                nc.sync.dma_start(wo_sb[i][:], woT[i * P:(i + 1) * P, :])
            ones_f = consts.tile([P, 8], f32, tag="onesf")
            nc.vector.memset(ones_f[:], 1.0)
            ones_bf = consts.tile([P, 8], bf16, tag="onesb")
            nc.vector.tensor_copy(ones_bf[:], ones_f[:])
            eps_sb = consts.tile([1, 1], f32, tag="eps")
            nc.vector.memset(eps_sb[:], EPS)
            ones_r = consts.tile([P, 1], f32r, tag="onesr")
            nc.vector.tensor_copy(ones_r[:], ones_f[:, 0:1])
            onesw_f = consts.tile([1, P], f32, tag="oneswf")
            nc.vector.memset(onesw_f[:], 1.0)
            h1sel = consts.tile([1, P], f32r, tag="h1sel")
            nc.vector.tensor_copy(h1sel[:], onesw_f[:])

            _stk = ExitStack()
            attnpool = _stk.enter_context(tc.tile_pool(name="attn", bufs=1))
            _qkstk = ExitStack()
            qkpool = _qkstk.enter_context(tc.tile_pool(name="qk", bufs=1))
            vapool = _qkstk.enter_context(tc.tile_pool(name="vaug", bufs=1))

            qT = [qkpool.tile([P, S], bf16, tag=f"qT{i}", name=f"qT{i}")
                  for i in range(4)]
            kT = [qkpool.tile([P, S], bf16, tag=f"kT{i}", name=f"kT{i}")
                  for i in range(4)]
            va = [vapool.tile([P, 8 * 65], bf16, tag=f"va{i}", name=f"va{i}")
                  for i in range(8)]
            attnT = [attnpool.tile([P, S], bf16, tag=f"at{i}", name=f"at{i}")
                     for i in range(4)]

            # [D-half][qc][512 D rows][512 seq]; each RS half contiguous
            po_dram = dram.tile([2, 2, 512, 512], bf16)
            rs_dram = dram.tile([2, 512, 512], bf16)

            # ---- phase 1+2: load xT / weights, project v then q,k ----
            _xwstk = ExitStack()
            xw = _xwstk.enter_context(tc.tile_pool(name="xw", bufs=1))
            with (
                tc.tile_pool(name="proj_ps", bufs=6, space="PSUM") as pps,
                tc.tile_pool(name="rtmp", bufs=4) as rtmp,
            ):
                xt = [xw.tile([P, S], bf16, tag=f"x{i}", name=f"x{i}")
                      for i in range(8)]
                cos_sb = xw.tile([P, S], f32, tag="cos")
                sin_sb = xw.tile([P, S], f32, tag="sin")
                nc.sync.dma_start(cos_sb[:], cosT[:])
                nc.sync.dma_start(sin_sb[:], sinS[:])
                wq_sb = [xw.tile([P, 512], bf16, tag=f"wq{i}", name=f"wqs{i}")
                         for i in range(8)]
                wk_sb = [xw.tile([P, 512], bf16, tag=f"wk{i}", name=f"wks{i}")
                         for i in range(8)]
                wv_sb = [xw.tile([P, 512], bf16, tag=f"wv{i}", name=f"wvs{i}")
                         for i in range(8)]
                for i in range(8):
                    nc.sync.dma_start(xt[i][:], xT[i * P:(i + 1) * P, :])
                    nc.sync.dma_start(wv_sb[i][:], wvT[i * P:(i + 1) * P, :])
                    nc.sync.dma_start(wq_sb[i][:], wqT[i * P:(i + 1) * P, :])
                    nc.sync.dma_start(wk_sb[i][:], wkT[i * P:(i + 1) * P, :])

                def project_v(st, pool=None, tag="ps"):
                    ps = (pool or pps).tile([P, 512], f32, tag=tag,
                                            name="ps")
                    for kd in range(8):
                        nc.tensor.matmul(
                            ps[:],
                            lhsT=xt[kd][:, st * P:(st + 1) * P],
                            rhs=wv_sb[kd][:],
                            start=(kd == 0), stop=(kd == 7))
                    va3 = va[st][:].rearrange("p (h e) -> p h e", h=8, e=65)
                    nc.vector.tensor_copy(
                        va3[:, :, 0:64],
                        ps[:].rearrange("p (h e) -> p h e", h=8, e=64))
                    nc.vector.tensor_copy(
                        va3[:, :, 64:65],
                        ones_bf[:].rearrange("p (h o) -> p h o", o=1))

                def project_qk(mt):
                    # both seq chunks share each lhsT slice
                    for wsb, dstT in ((wq_sb, qT), (wk_sb, kT)):
                        ps0 = pps.tile([P, 512], f32, tag="ps", name="ps")
                        ps1 = pps.tile([P, 512], f32, tag="ps", name="ps")
                        for kd in range(8):
                            lh = wsb[kd][:, mt * P:(mt + 1) * P]
                            nc.tensor.matmul(ps0[:], lhsT=lh,
                                             rhs=xt[kd][:, 0:512],
                                             start=(kd == 0), stop=(kd == 7))
                            nc.tensor.matmul(ps1[:], lhsT=lh,
                                             rhs=xt[kd][:, 512:1024],
                                             start=(kd == 0), stop=(kd == 7))
                        for nch, ps in ((0, ps0), (1, ps1)):
                            n0 = nch * 512
                            dst = dstT[mt][:, n0:n0 + 512]
                            tmp = rtmp.tile([P, 512], f32, tag="rt",
                                            name="rt")
                            nc.vector.stream_shuffle(tmp[:], ps[:], SWAP16)
                            nc.vector.tensor_tensor(
                                dst, ps[:], cos_sb[:, n0:n0 + 512], Alu.mult)
                            tmp2 = rtmp.tile([P, 512], bf16, tag="rt2",
                                             name="rt2")
                            nc.vector.tensor_tensor(
                                tmp2[:], tmp[:], sin_sb[:, n0:n0 + 512],
                                Alu.mult)
                            nc.gpsimd.tensor_tensor(dst, dst, tmp2[:],
                                                    Alu.add)

                # va[4..7] (k/v seq 512-1023, first used by qc1 kt>=4) are
                # deferred into the qc0 attention window as dense
                # full-array PE work that keeps the HAM un-throttled.
                for mt in range(4):
                    project_qk(mt)
                    project_v(mt)

            # ---- phase 3: differential attention ---------------------
            with (
                tc.tile_pool(name="st_ps", bufs=2, space="PSUM") as stp,
                tc.tile_pool(name="pv_ps", bufs=2, space="PSUM") as pvp,
                tc.tile_pool(name="ms_ps", bufs=1, space="PSUM") as msp,
                tc.tile_pool(name="bc_ps", bufs=1, space="PSUM") as bcp,
                tc.tile_pool(name="epool", bufs=3) as epool,
                tc.tile_pool(name="apool", bufs=3) as apool,
                tc.tile_pool(name="zpool", bufs=2) as zpool,
                tc.tile_pool(name="post", bufs=2) as post,
            ):
                zpt = {}
                ms = {}
                aw = {}

                def attn_pass(qc, pt, h):
                    # one head (2pt+h): branch pair gq=(2h, 2h+1).
                    # scores/exp for kt+1 are emitted BEFORE pv of kt so the
                    # strict-FIFO PE queue always has independent matmuls
                    # ahead of the exp-gated pv pair.
                    q0 = qc * 512
                    kts = _kts(qc)
                    last_kt = kts[-1][0]
                    pvA = pvp.tile([65, 512], f32, tag="pv", name="pvA")
                    pvB = pvp.tile([65, 512], f32, tag="pv", name="pvB")
                    hv = 2 * pt + h
                    es = {}

                    def emit_scores(kt, off):
                        j0 = 0 if off is None else off
                        st = stp.tile([P, 1024], f32, tag="st", name="st")
                        st3 = st[:].rearrange("p (g n) -> p g n", g=2, n=512)
                        for gi, g in enumerate((2 * h, 2 * h + 1)):
                            nc.tensor.matmul(
                                st3[:, gi, j0:],
                                lhsT=kT[pt][g * 32:(g + 1) * 32,
                                            kt * P:(kt + 1) * P],
                                rhs=qT[pt][g * 32:(g + 1) * 32,
                                           q0 + j0:q0 + 512],
                                start=True, stop=True,
                                tile_position=(g * 32, 0))
                        e = epool.tile([P, 1024], bf16, tag="e", name="e")
                        e3 = e[:].rearrange("p (g n) -> p g n", g=2, n=512)
                        nc.scalar.activation(e3[:, :, j0:], st3[:, :, j0:],
                                             Act.Exp, scale=SCALE)
                        if off is not None:
                            nc.vector.tensor_tensor(
                                e3[:, :, j0:j0 + P], e3[:, :, j0:j0 + P],
                                md_sb[:].rearrange("p (g n) -> p g n", g=2),
                                Alu.mult)
                        es[kt] = e3

                    def emit_pv(kt, off):
                        j0 = 0 if off is None else off
                        e3 = es.pop(kt)
                        nc.tensor.matmul(
                            pvA[:, j0:], lhsT=va[kt][:, hv * 65:hv * 65 + 65],
                            rhs=e3[:, 0, j0:],
                            start=(kt == 0), stop=(kt == last_kt))
                        nc.tensor.matmul(
                            pvB[:, j0:], lhsT=va[kt][:, hv * 65:hv * 65 + 65],
                            rhs=e3[:, 1, j0:],
                            start=(kt == 0), stop=(kt == last_kt))

                    emit_scores(*kts[0])
                    for i, (kt, off) in enumerate(kts):
                        if i + 1 < len(kts):
                            emit_scores(*kts[i + 1])
                        emit_pv(kt, off)
                    # evict: A rows (bf16) + Z rows at 32-aligned partitions.
                    # The last pass splits across DVE and the (by then idle)
                    # scalar engine to shorten the tail before wo/RS.
                    last = (qc == 1 and pt == 3)
                    a1, a2 = aw[(qc, pt)]
                    nc.vector.tensor_copy(a1[64 * h:64 * h + 64, :],
                                          pvA[0:64, :])
                    (nc.scalar.copy if last else nc.vector.tensor_copy)(
                        a2[64 * h:64 * h + 64, :], pvB[0:64, :])
                    z = zpt[(qc, pt)]
                    (nc.scalar.copy if last else nc.vector.tensor_copy)(
                        z[32 * h:32 * h + 1, :], pvA[64:65, :])
                    nc.vector.tensor_copy(z[64 + 32 * h:65 + 32 * h, :],
                                          pvB[64:65, :])

                def attn_pt(qc, pt):
                    if (qc, pt) not in aw:
                        aw[(qc, pt)] = (
                            apool.tile([P, 512], bf16, tag="a1", name="a1",
                                       bufs=5),
                            apool.tile([P, 512], bf16, tag="a2", name="a2",
                                       bufs=5))
                        z = zpool.tile([P, 512], f32r, tag="zpt", name="zpt",
                                       bufs=5)
                        nc.vector.memset(z[:].bitcast(f32), 0.0)
                        zpt[(qc, pt)] = z
                    attn_pass(qc, pt, 0)
                    attn_pass(qc, pt, 1)

                def emit_posw(qc, pt):
                    # posw = Z2*A1 - lam*Z1*A2 ; sq = posw^2/8 ; ms rows.
                    # The broadcasts read this pt's zpt directly (K=128
                    # selector) so posw pipelines with the next pt's pass.
                    a1, a2 = aw[(qc, pt)]
                    z = zpt[(qc, pt)]
                    bc2 = bcp.tile([P, 512], f32, tag="bc", name="bc2")
                    nc.tensor.matmul(bc2[:], lhsT=hz2_sb[:],
                                     rhs=z[:], start=True, stop=True)
                    t1 = post.tile([P, 512], f32r, tag="t1")
                    nc.vector.tensor_tensor(t1[:], a1[:], bc2[:], Alu.mult)
                    bc1 = bcp.tile([P, 512], f32, tag="bc", name="bc1")
                    nc.tensor.matmul(bc1[:], lhsT=hz1l_sb[:],
                                     rhs=z[:], start=True, stop=True)
                    t2 = post.tile([P, 512], f32r, tag="t2")
                    nc.vector.tensor_tensor(t2[:], a2[:], bc1[:], Alu.mult)
                    posw = post.tile([P, 512], f32r, tag="posw", bufs=5,
                                     name="posw")
                    nc.vector.tensor_tensor(posw[:], t1[:], t2[:],
                                            Alu.subtract)
                    sq = post.tile([P, 512], f32r, tag="sq", name="sq")
                    nc.vector.scalar_tensor_tensor(
                        sq[:], in0=posw[:], scalar=0.125, in1=posw[:],
                        op0=Alu.mult, op1=Alu.mult)
                    nc.tensor.matmul(ms[qc][:],
                                     lhsT=hselq_sb[:, 8 * pt:8 * pt + 8],
                                     rhs=sq[:], start=(pt == 0),
                                     stop=(pt == 3))
                    return posw

                def emit_rstd(qc):
                    # rstd rows = (1-li)*rsqrt(ms), [8,512]; the reference's
                    # +eps*(z1*z2)^2 term is ~4e-5 relative to ms and the
                    # (1-li) factor folds into the Rsqrt input scale.
                    srt = zpool.tile([8, 512], f32, tag="srt", name="srt")
                    nc.scalar.activation(srt[:], ms[qc][0:8, :], Act.Sqrt,
                                         scale=1.0 / (1.0 - LAMBDA_INIT) ** 2)
                    rstd = zpool.tile([8, 512], f32, tag="rstd", name="rstd")
                    nc.vector.reciprocal_approx_fast(rstd[:], srt[:])
                    rstr = zpool.tile([8, 512], f32r, tag="rstr",
                                      name="rstr")
                    nc.vector.tensor_copy(rstr[:], rstd[:])
                    return rstr[:]

                def emit_apply(qc, pt, posw, rstd):
                    q0 = qc * 512
                    bcr = bcp.tile([P, 512], f32, tag="bc", name="bcr")
                    nc.tensor.matmul(bcr[:],
                                     lhsT=hrstd_sb[:, pt * P:(pt + 1) * P],
                                     rhs=rstd, start=True, stop=True)
                    nc.vector.tensor_tensor(attnT[pt][:, q0:q0 + 512],
                                            posw[:], bcr[:], Alu.mult)

                poswq = {}
                for qc in range(2):
                    ms[qc] = msp.tile([8, 512], f32, tag="ms", name="ms")
                vdefer = {0: (4, 5), 1: (6,), 2: (7,), 3: ()}
                for pt in range(4):
                    attn_pt(0, pt)
                    for st in vdefer[pt]:
                        project_v(st, pool=bcp, tag="bc")
                    if pt > 0:
                        poswq[(0, pt - 1)] = emit_posw(0, pt - 1)
                # ---- phase 4 interleaved: wo by D-half + pair RS -----
                with tc.tile_pool(name="po_sb", bufs=4) as posb:
                    def emit_wo(dh, qc, pool=None):
                        q0 = qc * 512
                        for mo in range(4 * dh, 4 * dh + 4):
                            if pool is None:
                                pst = bcp.tile([P, 512], f32, tag="bc",
                                               name="wops")
                                ps = pst[:]
                            else:
                                pst = pool.tile([P, 1024], f32, tag="st",
                                                name="wops")
                                ps = pst[:, 0:512]
                            for kc in range(4):
                                nc.tensor.matmul(
                                    ps,
                                    lhsT=wo_sb[kc][:, mo * P:(mo + 1) * P],
                                    rhs=attnT[kc][:, q0:q0 + 512],
                                    start=(kc == 0), stop=(kc == 3))
                            po = posb.tile([P, 512], bf16, tag="po")
                            nc.vector.tensor_copy(po[:], ps)
                            nc.sync.dma_start(
                                po_dram[dh, qc,
                                        (mo % 4) * P:(mo % 4 + 1) * P, :],
                                po[:])

                    for pt in range(4):
                        attn_pt(1, pt)
                        if pt == 0:
                            poswq[(0, 3)] = emit_posw(0, 3)
                        else:
                            poswq[(1, pt - 1)] = emit_posw(1, pt - 1)
                        if pt == 1:
                            rstd0 = emit_rstd(0)
                            for p2 in range(4):
                                emit_apply(0, p2, poswq[(0, p2)], rstd0)
                            emit_wo(0, 0)
                        if pt == 2:
                            emit_wo(1, 0)
                    poswq[(1, 3)] = emit_posw(1, 3)
                    rstd1 = emit_rstd(1)
                    for p2 in range(4):
                        emit_apply(1, p2, poswq[(1, p2)], rstd1)
                    emit_wo(0, 1, pool=stp)
                    nc.gpsimd.collective_compute(
                        "ReduceScatter",
                        mybir.AluOpType.add,
                        replica_groups=[[0, 1], [2, 3], [4, 5], [6, 7]],
                        ins=[po_dram[0].opt()],
                        outs=[rs_dram[0].opt()],
                    )
                    emit_wo(1, 1, pool=stp)
                    nc.gpsimd.collective_compute(
                        "ReduceScatter",
                        mybir.AluOpType.add,
                        replica_groups=[[0, 1], [2, 3], [4, 5], [6, 7]],
                        ins=[po_dram[1].opt()],
                        outs=[rs_dram[1].opt()],
                    )

            _xwstk.close()
            _qkstk.close()
            _stk.close()

            # ---- phase 5: FFN + residual + final RMS on seq shard ----
            with (
                tc.tile_pool(name="aT", bufs=1) as atp,
                tc.tile_pool(name="h1", bufs=1) as h1p,
                tc.tile_pool(name="w1p", bufs=9) as w1p,
                tc.tile_pool(name="w2p", bufs=3) as w2p,
                tc.tile_pool(name="yT", bufs=1) as ytp,
                tc.tile_pool(name="fin", bufs=2) as finp,
                tc.tile_pool(name="sm2", bufs=1) as sm2,
            ):
                aTr = [atp.tile([P, 512], bf16, tag=f"ar{i}", name=f"ar{i}")
                       for i in range(8)]
                for i in range(8):
                    nc.sync.dma_start(
                        aTr[i][:],
                        rs_dram[i // 4, (i % 4) * P:(i % 4 + 1) * P, :])

                h1 = [h1p.tile([P, 512], bf16, tag=f"h1_{i}", name=f"h1_{i}")
                      for i in range(32)]
                with tc.tile_pool(name="h1_ps", bufs=8, space="PSUM") as h1ps:
                    wts = {}
                    pss = {}

                    def h1_first(mf):
                        wt = w1p.tile([P, 1024], bf16, tag="w1t", name="w1t")
                        nc.sync.dma_start(wt[:], w1s[mf, :, :])
                        ps = h1ps.tile([P, 512], f32, tag="h1ps",
                                       name="h1ps")
                        for kd in range(4):
                            nc.tensor.matmul(
                                ps[:], lhsT=wt[:, kd * P:(kd + 1) * P],
                                rhs=aTr[kd][:], start=(kd == 0), stop=False)
                        wts[mf], pss[mf] = wt, ps

                    def h1_second(mf):
                        wt, ps = wts.pop(mf), pss.pop(mf)
                        for kd in range(4, 8):
                            nc.tensor.matmul(
                                ps[:], lhsT=wt[:, kd * P:(kd + 1) * P],
                                rhs=aTr[kd][:], start=False, stop=(kd == 7))
                        nc.scalar.activation(h1[mf][:], ps[:], Act.Relu,
                                             bias=b1_sb[:, mf:mf + 1])

                    for mf in range(8):
                        h1_first(mf)
                    for mf in range(32):
                        h1_second(mf)
                        if mf + 8 < 32:
                            h1_first(mf + 8)

                # h2: 8 persistent PSUM accumulators, stream w2 tiles
                yt = [ytp.tile([P, 512], f32, tag=f"y{i}", name=f"y{i}")
                      for i in range(8)]
                with tc.tile_pool(name="h2_ps", bufs=1, space="PSUM") as h2ps:
                    ps8 = [h2ps.tile([P, 512], f32, tag=f"h2_{mo}",
                                     name=f"h2_{mo}") for mo in range(8)]
                    for kf in range(32):
                        wt2 = w2p.tile([P, 1024], bf16, tag="w2t",
                                       name="w2t")
                        nc.sync.dma_start(wt2[:], w2T[kf * P:(kf + 1) * P, :])
                        for mo in range(8):
                            nc.tensor.matmul(
                                ps8[mo][:], lhsT=wt2[:, mo * P:(mo + 1) * P],
                                rhs=h1[kf][:], start=(kf == 0),
                                stop=(kf == 31))
                    for mo in range(8):
                        nc.vector.scalar_tensor_tensor(
                            yt[mo][:], in0=ps8[mo][:],
                            scalar=b2_sb[:, mo:mo + 1], in1=aTr[mo][:],
                            op0=Alu.add, op1=Alu.add)

                with tc.tile_pool(name="rms_ps", bufs=1,
                                  space="PSUM") as rmsps:
                    ms_ps = rmsps.tile([P, 512], f32, tag="rmsps",
                                       name="rmsps")
                    for mo in range(8):
                        sq = finp.tile([P, 512], f32r, tag="fsq", name="fsq")
                        nc.scalar.activation(sq[:], yt[mo][:], Act.Square)
                        nc.tensor.matmul(ms_ps[0:1, :], lhsT=ones_r[:],
                                         rhs=sq[:], start=(mo == 0),
                                         stop=(mo == 7))
                    srt = sm2.tile([1, 512], f32, tag="fsrt")
                    nc.scalar.activation(srt[:], ms_ps[0:1, :], Act.Sqrt,
                                         scale=1.0 / 1024.0, bias=eps_sb[:])
                    rstd = sm2.tile([1, 512], f32, tag="frstd")
                    nc.vector.reciprocal_approx_fast(rstd[:], srt[:])
                    rstr = sm2.tile([1, 512], f32r, tag="frstr")
                    nc.vector.tensor_copy(rstr[:], rstd[:])
                    bcr = rmsps.tile([P, 512], f32, tag="fbc", name="fbc")
                    nc.tensor.matmul(bcr[:], lhsT=h1sel[:], rhs=rstr[:],
                                     start=True, stop=True)
                    for mo in range(8):
                        ot = finp.tile([P, 512], f32, tag="fot", name="fot")
                        nc.vector.scalar_tensor_tensor(
                            ot[:], in0=yt[mo][:], scalar=rw_sb[:, mo:mo + 1],
                            in1=bcr[:], op0=Alu.mult, op1=Alu.mult)
                        nc.sync.dma_start(outT[mo * P:(mo + 1) * P, :], ot[:])

    nc.compile()
    return nc


def _host_prep(inputs):
    import ml_dtypes
    bfloat16 = ml_dtypes.bfloat16
    x = np.asarray(inputs["x"], dtype=np.float32)
    Wq = np.asarray(inputs["Wq"], dtype=np.float32)
    Wk = np.asarray(inputs["Wk"], dtype=np.float32)
    Wv = np.asarray(inputs["Wv"], dtype=np.float32)
    Wo = np.asarray(inputs["Wo"], dtype=np.float32)
    W1 = np.asarray(inputs["W1"], dtype=np.float32)
    b1 = np.asarray(inputs["b1"], dtype=np.float32)
    W2 = np.asarray(inputs["W2"], dtype=np.float32)
    b2 = np.asarray(inputs["b2"], dtype=np.float32)
    rmsw = np.asarray(inputs["rms_weight"], dtype=np.float32)
    lam = float(np.exp(np.dot(np.asarray(inputs["lambda_q1"], np.float64),
                              np.asarray(inputs["lambda_k1"], np.float64)))
                - np.exp(np.dot(np.asarray(inputs["lambda_q2"], np.float64),
                                np.asarray(inputs["lambda_k2"], np.float64)))
                + LAMBDA_INIT)

    half = HD // 2
    freqs = (1.0 / (10000.0 ** (np.arange(half, dtype=np.float32)
                                / np.float32(half)))).astype(np.float32)
    ang = (np.arange(S, dtype=np.float32)[:, None] * freqs[None, :])
    cos16 = np.cos(ang.astype(np.float32)).T.astype(np.float32)
    sin16 = np.sin(ang.astype(np.float32)).T.astype(np.float32)

    cosT = np.ascontiguousarray(
        np.tile(np.concatenate([cos16, cos16], 0), (4, 1)))
    sinS = np.ascontiguousarray(
        np.tile(np.concatenate([-sin16, sin16], 0), (4, 1))).astype(np.float32)
    perm32 = np.concatenate([np.arange(0, 32, 2), np.arange(1, 32, 2)])

    # multiplicative causal mask for the diag band, tiled for both branches
    md = (np.arange(128)[:, None] <= np.arange(128)[None, :])
    mdiag = np.ascontiguousarray(
        np.tile(md.astype(np.float32), (1, 2)).astype(bfloat16))

    # selector matmuls: bc[p,s] = sum_k lhsT[k,p] * zq[k,s]
    # zq rows 0..7 = Z1 (key 2pt+h), 8..15 = Z2
    hz1l = np.zeros((128, 128), np.float32)
    hz2 = np.zeros((128, 128), np.float32)
    hselq = np.zeros((128, 32), np.float32)
    hrstd = np.zeros((8, 512), np.float32)
    for h in range(2):
        # bc[p, s] = zpt[32h(p), s]; zpt row 32h = Z1(h), 64+32h = Z2(h)
        pcols = slice(64 * h, 64 * h + 64)
        hz1l[32 * h, pcols] = lam
        hz2[64 + 32 * h, pcols] = 1.0
    for pt in range(4):
        for h in range(2):
            rows = slice(64 * h, 64 * h + 64)
            cols = slice(pt * 128 + 64 * h, pt * 128 + 64 * h + 64)
            hselq[rows, 8 * pt + 2 * pt + h] = 0.125
            hrstd[2 * pt + h, cols] = 1.0

    b1c = np.ascontiguousarray(b1.reshape(32, 128).T)
    b2c = np.ascontiguousarray(b2.reshape(8, 128).T)
    rmswc = np.ascontiguousarray(rmsw.reshape(8, 128).T)
    # w1s[mf][p, kd*128+j] = W1.T[kd*128+p, mf*128+j]
    w1s = np.ascontiguousarray(
        W1.T.reshape(8, 128, 32, 128).transpose(2, 1, 0, 3)
        .reshape(32, 128, 1024).astype(bfloat16))
    w2T = np.ascontiguousarray(W2.T.astype(bfloat16))

    in_maps = []
    for c in range(NCORES):
        b, g = c // 2, c % 2
        chans = np.arange(g * 512, (g + 1) * 512)
        permed = np.concatenate(
            [c0 * 32 + perm32 for c0 in range(g * 16, (g + 1) * 16)])
        in_maps.append({
            "xT": np.ascontiguousarray(x[b].T.astype(bfloat16)),
            "wqT": np.ascontiguousarray(Wq[permed, :].T.astype(bfloat16)),
            "wkT": np.ascontiguousarray(Wk[permed, :].T.astype(bfloat16)),
            "wvT": np.ascontiguousarray(Wv[chans, :].T.astype(bfloat16)),
            "woT": np.ascontiguousarray(Wo[:, chans].T.astype(bfloat16)),
            "w1s": w1s, "w2T": w2T,
            "b1c": b1c, "b2c": b2c, "rmswc": rmswc,
            "cosT": cosT, "sinS": sinS,
            "mdiag": mdiag,
            "hz1l": hz1l, "hz2": hz2, "hselq": hselq, "hrstd": hrstd,
        })
    return in_maps


def kernel(**inputs):
    global LAST_RESULT
    from concourse.bass_utils import run_bass_kernel_spmd

    if "nc" not in _PROGRAM:
        _PROGRAM["nc"] = _build_program()
    nc = _PROGRAM["nc"]

    in_maps = _host_prep(inputs)
    trace = bool(int(os.environ.get("KERNEL_TRACE", "0")))
    res = run_bass_kernel_spmd(nc, in_maps, list(range(NCORES)), trace=trace)
    LAST_RESULT = res

    out = np.empty((B, S, D), np.float32)
    for c in range(NCORES):
        b, g = c // 2, c % 2
        out[b, g * 512:(g + 1) * 512, :] = res.results[c]["outT"].T
    return out
